# revision 40
# baseline (speedup 1.0000x reference)
"""MoE router (linear gate -> softmax -> top-8 indices) on 8 Trainium2 cores.

Strategy (data-parallel over tokens, W replicated):
  - Each core gets 2048 tokens. x is pre-transposed on the host so each core
    receives x^T [4096, 2048] — the PE needs the contraction dim (d_model) on
    partitions and fp32 has no DMA-transpose path, so transposing on-chip
    would double PE work.
  - Softmax is strictly monotonic, so top-k of softmax(logits) == top-k of
    logits; the softmax is skipped entirely.
  - The gate matmul runs in float32r (fp20: 1+8+11) which streams at 1
    cycle/row vs fp32's 4, using an exactly-compensated split:
        x = x_hi + x_lo,  W = w_hi + w_lo   (each half fp20-representable)
        logits = w_hi·x_hi + w_hi·x_lo + w_lo·x_hi   (fp32 PSUM)
    The dropped w_lo·x_lo term is O(2^-24) relative — fp32-level accuracy
    (validated on HW: max err 1.5e-7 vs fp32's 1.2e-7). W is split on the
    host; x is split on-chip (ACT rounds to f32r — engines round f32r
    MEMORY on both read and write, bit-identical to host RNE; DVE
    subtracts) so HBM traffic stays 4 bytes/element.
  - PE work is 2 passes per chunk, not 3: the stationary is [w_hi | w_lo]
    [128, 128], so pass A (moving x_hi) yields w_hi·x_hi in PSUM rows 0-63
    AND w_lo·x_hi in rows 64-127 from one moving stream; pass B (moving
    x_lo) uses only the w_hi half into rows 0-63. The two halves are summed
    after the tail transpose, where they sit in the free dim.
  - Streaming: 32 x 1 MiB DMAs (one 128-row contraction chunk each, 358 GB/s
    measured), per-chunk ACT/DVE split, PE accumulates 4 [128, 512] PSUM
    logit tiles across all 32 chunks. The first and last chunk split/matmul
    at 512-token strip granularity to shorten pipeline fill and drain
    (Tile-framework dependencies are tile-granular).
  - Top-8: PE-transpose the logit tiles to [128 tokens, 128], DVE-add the
    two 64-wide halves, then DVE Max8 / MaxIndex produce the 8 largest
    values and indices per token (descending, ties -> lowest index, matching
    jax.lax.top_k). Indices are staged in SBUF, one DMA per token group.
"""

import numpy as np

import concourse.bass as bass
import concourse.mybir as mybir
import concourse.tile as tile
from concourse import bacc
from concourse.bass_utils import run_bass_kernel_spmd
from concourse.masks import make_identity

N_CORES = 8
N_TOKENS = 16384
D_MODEL = 4096
N_EXPERTS = 64
TOP_K = 8

TPC = N_TOKENS // N_CORES      # tokens per core (2048)
GROUP = 512                    # tokens per matmul (max 4-byte moving dim)
N_GROUPS = TPC // GROUP        # 4
N_CHUNK = D_MODEL // 128       # 32 contraction chunks
N_BLK = TPC // 128             # 16 x 128-token output blocks

F32 = mybir.dt.float32
F32R = mybir.dt.float32r
U32 = mybir.dt.uint32

_CACHE: dict = {}


def _build_program(
    x_bufs: int = 5,
    hilo_bufs: int = 4,
    fused_tail: bool = False,
    strip_first: bool = True,
    strip_last: bool = True,
    dual_ring: bool = False,
):
    nc = bacc.Bacc(
        "TRN2", target_bir_lowering=False, debug=False, num_devices=N_CORES
    )
    xt_d = nc.dram_tensor("xt", [D_MODEL, TPC], F32, kind="ExternalInput")
    # [w_hi | w_lo] packed on host: [128, 32, 128] with
    # [p, k, e]      = W_hi[e, k*128+p]  for e < 64
    # [p, k, 64+e]   = W_lo[e, k*128+p]
    ww_d = nc.dram_tensor(
        "ww", [128, N_CHUNK * 2 * N_EXPERTS], F32R, kind="ExternalInput"
    )
    # idx laid out [128 partitions, 16 blocks, 8] — host unpermutes to [2048, 8]
    idx_d = nc.dram_tensor("idx", [128, N_BLK * TOP_K], U32, kind="ExternalOutput")

    with tile.TileContext(nc) as tc:
        with (
            tc.tile_pool(name="const", bufs=1) as const_pool,
            tc.tile_pool(name="xin", bufs=x_bufs) as x_pool,
            tc.tile_pool(name="hi", bufs=hilo_bufs) as hi_pool,
            tc.tile_pool(name="lo", bufs=hilo_bufs) as lo_pool,
            tc.tile_pool(name="lg_ps", bufs=1, space="PSUM") as lg_ps_pool,
            tc.tile_pool(name="lt_ps", bufs=2, space="PSUM") as lt_ps_pool,
            tc.tile_pool(name="small", bufs=2 * 4) as small_pool,
        ):
            # stacked double identity [I64; I64]: as the moving operand of
            # the tail "transpose", it transposes AND sums the wh/wl halves
            # of the logit tiles in one PE pass.
            ident = const_pool.tile([128, 128], F32)
            make_identity(nc, ident[:])
            if fused_tail:
                ident2 = const_pool.tile([128, N_EXPERTS], F32)
                nc.vector.tensor_tensor(
                    ident2[:],
                    ident[:, :N_EXPERTS],
                    ident[:, N_EXPERTS:],
                    mybir.AluOpType.add,
                )
            # W DMA goes on the scalar HWDGE ring so x chunk 0 (sync ring)
            # isn't queued behind it.
            ww_sb = const_pool.tile([128, N_CHUNK, 2 * N_EXPERTS], F32R)
            ww_view = ww_d.ap().rearrange("p (k e) -> p k e", k=N_CHUNK)
            half = N_CHUNK // 2
            nc.scalar.dma_start(ww_sb[:, :half], ww_view[:, :half])
            nc.scalar.dma_start(ww_sb[:, half:], ww_view[:, half:])
            lg_ps = [
                lg_ps_pool.tile(
                    [2 * N_EXPERTS, GROUP], F32, name=f"lg{g}", tag=f"lg{g}"
                )
                for g in range(N_GROUPS)
            ]

            xt_view = xt_d.ap().rearrange("(k p) t -> p k t", p=128)
            for k in range(N_CHUNK):
                x_sb = x_pool.tile([128, TPC], F32)
                x_eng = nc.scalar if (dual_ring and k % 2 == 1) else nc.sync
                x_eng.dma_start(x_sb[:], xt_view[:, k, :])
                # hi = round_f32r(x) on ACT (engines round by the MEMORY
                # dtype: an f32r tile rounds on every engine read, so the
                # raw x tile must stay f32-declared and hi gets its own
                # f32r tile). lo = x - hi on DVE, write-rounded (exact).
                # Tile-framework deps are tile-granular, so the first and
                # last chunk use per-group STRIP TILES: chunk 0's group-0
                # matmul then starts after 1/4 of the split latency, and
                # chunk 31's pipeline drain shrinks the same way.
                edge = (k == 0 or k == N_CHUNK - 1) and os.environ.get(
                    "KERNEL_EDGE_STRIPS", "1"
                ) == "1"
                if edge:
                    hi_s, lo_s = [], []
                    for g in range(N_GROUPS):
                        ssl = slice(g * GROUP, (g + 1) * GROUP)
                        h = hi_pool.tile(
                            [128, GROUP], F32R, name=f"hs{k}_{g}", tag=f"hs{g}",
                            bufs=2,
                        )
                        nc.scalar.copy(h[:], x_sb[:, ssl])
                        l = lo_pool.tile(
                            [128, GROUP], F32R, name=f"ls{k}_{g}", tag=f"ls{g}",
                            bufs=2,
                        )
                        nc.vector.tensor_tensor(
                            l[:], x_sb[:, ssl], h[:].bitcast(F32),
                            mybir.AluOpType.subtract,
                        )
                        hi_s.append(h)
                        lo_s.append(l)
                    his = lambda g: hi_s[g][:]
                    los = lambda g: lo_s[g][:]
                else:
                    hi = hi_pool.tile([128, TPC], F32R, tag="hi")
                    nc.scalar.copy(hi[:], x_sb[:])
                    lo = lo_pool.tile([128, TPC], F32R, tag="lo")
                    nc.vector.tensor_tensor(
                        lo[:], x_sb[:], hi[:].bitcast(F32),
                        mybir.AluOpType.subtract,
                    )
                    his = lambda g, t=hi: t[:, g * GROUP : (g + 1) * GROUP]
                    los = lambda g, t=lo: t[:, g * GROUP : (g + 1) * GROUP]
                # pass A: [w_hi|w_lo]·hi -> all 128 PSUM rows
                # pass B: w_hi·lo -> rows 0-63 only
                # A-block before B-block: pass A depends only on the ACT
                # rounding, so the PE starts each chunk without waiting for
                # the DVE subtract. chunk 0 OPENs each tile with a
                # full-tile start; chunk 31's A-block runs last and CLOSEs
                # with a full-tile stop.
                if k < N_CHUNK - 1:
                    for g in range(N_GROUPS):
                        nc.tensor.matmul(
                            lg_ps[g][:], ww_sb[:, k], his(g),
                            start=(k == 0), stop=False,
                        )
                    for g in range(N_GROUPS):
                        nc.tensor.matmul(
                            lg_ps[g][: N_EXPERTS], ww_sb[:, k, :N_EXPERTS],
                            los(g), start=False, stop=False,
                        )
                else:
                    for g in range(N_GROUPS):
                        nc.tensor.matmul(
                            lg_ps[g][: N_EXPERTS], ww_sb[:, k, :N_EXPERTS],
                            los(g), start=False, stop=False,
                        )
                    for g in range(N_GROUPS):
                        nc.tensor.matmul(
                            lg_ps[g][:], ww_sb[:, k], his(g),
                            start=False, stop=True,
                        )

            idx_view = idx_d.ap().rearrange("p (b k) -> p b k", b=N_BLK)
            n_gb = GROUP // 128
            for g in range(N_GROUPS):
                lg_sb = small_pool.tile([2 * N_EXPERTS, GROUP], F32, tag="lgsb")
                nc.scalar.copy(lg_sb[:], lg_ps[g][:])
                idx_g = small_pool.tile([128, n_gb, TOP_K], U32, tag="idxg")
                for b in range(n_gb):
                    if fused_tail:
                        # fused transpose+sum: lg_slice.T @ [I64; I64]
                        lt_ps = lt_ps_pool.tile([128, N_EXPERTS], F32)
                        nc.tensor.matmul(
                            lt_ps[:],
                            lg_sb[:, b * 128 : (b + 1) * 128],
                            ident2[:],
                            start=True, stop=True,
                        )
                        lt_in = lt_ps
                    else:
                        lt_ps = lt_ps_pool.tile([128, 2 * N_EXPERTS], F32)
                        nc.tensor.transpose(
                            lt_ps[:],
                            lg_sb[:, b * 128 : (b + 1) * 128],
                            ident[:],
                        )
                        lt_h = small_pool.tile([128, N_EXPERTS], F32, tag="lth")
                        nc.scalar.copy(lt_h[:], lt_ps[:, :N_EXPERTS])
                        lt_in = small_pool.tile([128, N_EXPERTS], F32, tag="ltsb")
                        nc.vector.tensor_tensor(
                            lt_in[:],
                            lt_h[:],
                            lt_ps[:, N_EXPERTS:],
                            mybir.AluOpType.add,
                        )
                    vals = small_pool.tile([128, TOP_K], F32, tag="vals")
                    nc.vector.max(vals[:], lt_in[:])
                    nc.vector.max_index(idx_g[:, b, :], vals[:], lt_in[:])
                nc.sync.dma_start(
                    idx_view[:, g * n_gb : (g + 1) * n_gb, :], idx_g[:]
                )

    nc.compile()
    return nc


def _get_program():
    if "nc" not in _CACHE:
        _CACHE["nc"] = _build_program()
    return _CACHE["nc"]


def _round_f32r(a: np.ndarray) -> np.ndarray:
    """Round fp32 -> fp20 (1+8+11 float32r), RNE, kept as fp32 bit pattern."""
    u = np.ascontiguousarray(a, dtype=np.float32).view(np.uint32)
    low = u & np.uint32(0x00000FFF)
    base = u & np.uint32(0xFFFFF000)
    half = np.uint32(0x800)
    lsb = (u >> np.uint32(12)) & np.uint32(1)
    round_up = (low > half) | ((low == half) & (lsb == 1))
    return (base + np.where(round_up, np.uint32(0x1000), np.uint32(0))).view(
        np.float32
    )


def _pack_ww(W: np.ndarray) -> np.ndarray:
    # [64, 4096] -> [128, 32*128]: [p, k*128+e] = W_hi[e, k*128+p],
    #                              [p, k*128+64+e] = W_lo[e, k*128+p]
    wt = (
        W.astype(np.float32, copy=False)
        .T.reshape(N_CHUNK, 128, N_EXPERTS)
        .transpose(1, 0, 2)
    )  # [128, 32, 64]
    wh = _round_f32r(wt)
    wl = _round_f32r(wt - wh)
    ww = np.concatenate([wh.reshape(128, N_CHUNK, N_EXPERTS),
                         wl.reshape(128, N_CHUNK, N_EXPERTS)], axis=2)
    return np.ascontiguousarray(ww.reshape(128, N_CHUNK * 2 * N_EXPERTS))


def _make_in_maps(x: np.ndarray, W: np.ndarray) -> list:
    x = np.asarray(x, dtype=np.float32)
    ww = _pack_ww(W)
    return [
        {
            "xt": np.ascontiguousarray(x[c * TPC : (c + 1) * TPC].T),
            "ww": ww,
        }
        for c in range(N_CORES)
    ]


def kernel(x: np.ndarray, W: np.ndarray) -> np.ndarray:
    nc = _get_program()
    in_maps = _make_in_maps(x, W)
    res = run_bass_kernel_spmd(nc, in_maps, core_ids=list(range(N_CORES)))
    out = np.concatenate(
        [
            res.results[c]["idx"]
            .reshape(128, N_BLK, TOP_K)
            .transpose(1, 0, 2)
            .reshape(TPC, TOP_K)
            for c in range(N_CORES)
        ],
        axis=0,
    )
    return out.astype(np.int32)


# revision 42
# speedup vs baseline: 1.1638x; 1.1638x over previous
"""MoE router (linear gate -> softmax -> top-8 indices) on 8 Trainium2 cores.

Strategy (data-parallel over tokens, W replicated):
  - Each core gets 2048 tokens. x is pre-transposed on the host so each core
    receives x^T [4096, 2048] — the PE needs the contraction dim (d_model) on
    partitions and fp32 has no DMA-transpose path, so transposing on-chip
    would double PE work.
  - Softmax is strictly monotonic, so top-k of softmax(logits) == top-k of
    logits; the softmax is skipped entirely.
  - The gate matmul runs in float32r (fp20: 1+8+11) which streams at 1
    cycle/row vs fp32's 4, using an exactly-compensated split:
        x = x_hi + x_lo,  W = w_hi + w_lo   (each half fp20-representable)
        logits = w_hi·x_hi + w_hi·x_lo + w_lo·x_hi   (fp32 PSUM)
    The dropped w_lo·x_lo term is O(2^-24) relative — fp32-level accuracy
    (validated on HW: max err 1.5e-7 vs fp32's 1.2e-7). W is split on the
    host; x is split on-chip (ACT rounds to f32r — engines round f32r
    MEMORY on both read and write, bit-identical to host RNE; DVE
    subtracts) so HBM traffic stays 4 bytes/element.
  - PE work is 2 passes per chunk, not 3: the stationary is [w_hi | w_lo]
    [128, 128], so pass A (moving x_hi) yields w_hi·x_hi in PSUM rows 0-63
    AND w_lo·x_hi in rows 64-127 from one moving stream; pass B (moving
    x_lo) uses only the w_hi half into rows 0-63. The two halves are summed
    after the tail transpose, where they sit in the free dim.
  - Streaming: 32 x 1 MiB DMAs (one 128-row contraction chunk each, 358 GB/s
    measured), per-chunk ACT/DVE split, PE accumulates 4 [128, 512] PSUM
    logit tiles across all 32 chunks. The first and last chunk split/matmul
    at 512-token strip granularity to shorten pipeline fill and drain
    (Tile-framework dependencies are tile-granular).
  - Top-8: PE-transpose the logit tiles to [128 tokens, 128], DVE-add the
    two 64-wide halves, then DVE Max8 / MaxIndex produce the 8 largest
    values and indices per token (descending, ties -> lowest index, matching
    jax.lax.top_k). Indices are staged in SBUF, one DMA per token group.
"""

import numpy as np

import concourse.bass as bass
import concourse.mybir as mybir
import concourse.tile as tile
from concourse import bacc
from concourse.bass_utils import run_bass_kernel_spmd
from concourse.masks import make_identity

N_CORES = 8
N_TOKENS = 16384
D_MODEL = 4096
N_EXPERTS = 64
TOP_K = 8

TPC = N_TOKENS // N_CORES      # tokens per core (2048)
GROUP = 512                    # tokens per matmul (max 4-byte moving dim)
N_GROUPS = TPC // GROUP        # 4
N_CHUNK = D_MODEL // 128       # 32 contraction chunks
N_BLK = TPC // 128             # 16 x 128-token output blocks

F32 = mybir.dt.float32
F32R = mybir.dt.float32r
U32 = mybir.dt.uint32

_CACHE: dict = {}


def _build_program(
    x_bufs: int = 5,
    hilo_bufs: int = 4,
    fused_tail: bool = False,
    strip_first: bool = True,
    strip_last: bool = True,
    dual_ring: bool = False,
    dma_pair: bool = False,
):
    nc = bacc.Bacc(
        "TRN2", target_bir_lowering=False, debug=False, num_devices=N_CORES
    )
    xt_d = nc.dram_tensor("xt", [D_MODEL, TPC], F32, kind="ExternalInput")
    # [w_hi | w_lo] packed on host: [128, 32, 128] with
    # [p, k, e]      = W_hi[e, k*128+p]  for e < 64
    # [p, k, 64+e]   = W_lo[e, k*128+p]
    ww_d = nc.dram_tensor(
        "ww", [128, N_CHUNK * 2 * N_EXPERTS], F32R, kind="ExternalInput"
    )
    # idx laid out [128 partitions, 16 blocks, 8] — host unpermutes to [2048, 8]
    idx_d = nc.dram_tensor("idx", [128, N_BLK * TOP_K], U32, kind="ExternalOutput")

    with tile.TileContext(nc) as tc:
        with (
            tc.tile_pool(name="const", bufs=1) as const_pool,
            tc.tile_pool(name="xin", bufs=x_bufs) as x_pool,
            tc.tile_pool(name="hi", bufs=hilo_bufs) as hi_pool,
            tc.tile_pool(name="lo", bufs=hilo_bufs) as lo_pool,
            tc.tile_pool(name="lg_ps", bufs=1, space="PSUM") as lg_ps_pool,
            tc.tile_pool(name="lt_ps", bufs=2, space="PSUM") as lt_ps_pool,
            tc.tile_pool(name="small", bufs=2 * 4) as small_pool,
        ):
            # stacked double identity [I64; I64]: as the moving operand of
            # the tail "transpose", it transposes AND sums the wh/wl halves
            # of the logit tiles in one PE pass.
            ident = const_pool.tile([128, 128], F32)
            make_identity(nc, ident[:])
            if fused_tail:
                ident2 = const_pool.tile([128, N_EXPERTS], F32)
                nc.vector.tensor_tensor(
                    ident2[:],
                    ident[:, :N_EXPERTS],
                    ident[:, N_EXPERTS:],
                    mybir.AluOpType.add,
                )
            # W DMA goes on the scalar HWDGE ring so x chunk 0 (sync ring)
            # isn't queued behind it.
            ww_sb = const_pool.tile([128, N_CHUNK, 2 * N_EXPERTS], F32R)
            ww_view = ww_d.ap().rearrange("p (k e) -> p k e", k=N_CHUNK)
            half = N_CHUNK // 2
            nc.scalar.dma_start(ww_sb[:, :half], ww_view[:, :half])
            nc.scalar.dma_start(ww_sb[:, half:], ww_view[:, half:])
            lg_ps = [
                lg_ps_pool.tile(
                    [2 * N_EXPERTS, GROUP], F32, name=f"lg{g}", tag=f"lg{g}"
                )
                for g in range(N_GROUPS)
            ]

            xt_view = xt_d.ap().rearrange("(k p) t -> p k t", p=128)
            x_pair = None
            for k in range(N_CHUNK):
                if dma_pair:
                    if k % 2 == 0:
                        x_pair = x_pool.tile([128, 2, TPC], F32, tag="xp")
                        nc.sync.dma_start(
                            x_pair[:], xt_view[:, k : k + 2, :]
                        )
                    x_sb = x_pair[:, k % 2]
                else:
                    x_sb = x_pool.tile([128, TPC], F32)
                    x_eng = nc.scalar if (dual_ring and k % 2 == 1) else nc.sync
                    x_eng.dma_start(x_sb[:], xt_view[:, k, :])
                # hi = round_f32r(x) on ACT (engines round by the MEMORY
                # dtype: an f32r tile rounds on every engine read, so the
                # raw x tile must stay f32-declared and hi gets its own
                # f32r tile). lo = x - hi on DVE, write-rounded (exact).
                # Tile-framework deps are tile-granular, so the first and
                # last chunk use per-group STRIP TILES: chunk 0's group-0
                # matmul then starts after 1/4 of the split latency, and
                # chunk 31's pipeline drain shrinks the same way.
                edge = (k == 0 and strip_first) or (
                    k == N_CHUNK - 1 and strip_last
                )
                if edge:
                    hi_s, lo_s = [], []
                    for g in range(N_GROUPS):
                        ssl = slice(g * GROUP, (g + 1) * GROUP)
                        h = hi_pool.tile(
                            [128, GROUP], F32R, name=f"hs{k}_{g}", tag=f"hs{g}",
                            bufs=2,
                        )
                        nc.scalar.copy(h[:], x_sb[:, ssl])
                        l = lo_pool.tile(
                            [128, GROUP], F32R, name=f"ls{k}_{g}", tag=f"ls{g}",
                            bufs=2,
                        )
                        nc.vector.tensor_tensor(
                            l[:], x_sb[:, ssl], h[:].bitcast(F32),
                            mybir.AluOpType.subtract,
                        )
                        hi_s.append(h)
                        lo_s.append(l)
                    his = lambda g: hi_s[g][:]
                    los = lambda g: lo_s[g][:]
                else:
                    hi = hi_pool.tile([128, TPC], F32R, tag="hi")
                    nc.scalar.copy(hi[:], x_sb[:])
                    lo = lo_pool.tile([128, TPC], F32R, tag="lo")
                    nc.vector.tensor_tensor(
                        lo[:], x_sb[:], hi[:].bitcast(F32),
                        mybir.AluOpType.subtract,
                    )
                    his = lambda g, t=hi: t[:, g * GROUP : (g + 1) * GROUP]
                    los = lambda g, t=lo: t[:, g * GROUP : (g + 1) * GROUP]
                # pass A: [w_hi|w_lo]·hi -> all 128 PSUM rows
                # pass B: w_hi·lo -> rows 0-63 only
                # A-block before B-block: pass A depends only on the ACT
                # rounding, so the PE starts each chunk without waiting for
                # the DVE subtract. chunk 0 OPENs each tile with a
                # full-tile start; chunk 31's A-block runs last and CLOSEs
                # with a full-tile stop.
                if k < N_CHUNK - 1:
                    for g in range(N_GROUPS):
                        nc.tensor.matmul(
                            lg_ps[g][:], ww_sb[:, k], his(g),
                            start=(k == 0), stop=False,
                        )
                    for g in range(N_GROUPS):
                        nc.tensor.matmul(
                            lg_ps[g][: N_EXPERTS], ww_sb[:, k, :N_EXPERTS],
                            los(g), start=False, stop=False,
                        )
                else:
                    for g in range(N_GROUPS):
                        nc.tensor.matmul(
                            lg_ps[g][: N_EXPERTS], ww_sb[:, k, :N_EXPERTS],
                            los(g), start=False, stop=False,
                        )
                    for g in range(N_GROUPS):
                        nc.tensor.matmul(
                            lg_ps[g][:], ww_sb[:, k], his(g),
                            start=False, stop=True,
                        )

            idx_view = idx_d.ap().rearrange("p (b k) -> p b k", b=N_BLK)
            n_gb = GROUP // 128
            for g in range(N_GROUPS):
                lg_sb = small_pool.tile([2 * N_EXPERTS, GROUP], F32, tag="lgsb")
                nc.scalar.copy(lg_sb[:], lg_ps[g][:])
                idx_g = small_pool.tile([128, n_gb, TOP_K], U32, tag="idxg")
                for b in range(n_gb):
                    if fused_tail:
                        # fused transpose+sum: lg_slice.T @ [I64; I64]
                        lt_ps = lt_ps_pool.tile([128, N_EXPERTS], F32)
                        nc.tensor.matmul(
                            lt_ps[:],
                            lg_sb[:, b * 128 : (b + 1) * 128],
                            ident2[:],
                            start=True, stop=True,
                        )
                        lt_in = lt_ps
                    else:
                        lt_ps = lt_ps_pool.tile([128, 2 * N_EXPERTS], F32)
                        nc.tensor.transpose(
                            lt_ps[:],
                            lg_sb[:, b * 128 : (b + 1) * 128],
                            ident[:],
                        )
                        lt_h = small_pool.tile([128, N_EXPERTS], F32, tag="lth")
                        nc.scalar.copy(lt_h[:], lt_ps[:, :N_EXPERTS])
                        lt_in = small_pool.tile([128, N_EXPERTS], F32, tag="ltsb")
                        nc.vector.tensor_tensor(
                            lt_in[:],
                            lt_h[:],
                            lt_ps[:, N_EXPERTS:],
                            mybir.AluOpType.add,
                        )
                    vals = small_pool.tile([128, TOP_K], F32, tag="vals")
                    nc.vector.max(vals[:], lt_in[:])
                    nc.vector.max_index(idx_g[:, b, :], vals[:], lt_in[:])
                nc.sync.dma_start(
                    idx_view[:, g * n_gb : (g + 1) * n_gb, :], idx_g[:]
                )

    nc.compile()
    return nc


def _get_program():
    if "nc" not in _CACHE:
        _CACHE["nc"] = _build_program()
    return _CACHE["nc"]


def _round_f32r(a: np.ndarray) -> np.ndarray:
    """Round fp32 -> fp20 (1+8+11 float32r), RNE, kept as fp32 bit pattern."""
    u = np.ascontiguousarray(a, dtype=np.float32).view(np.uint32)
    low = u & np.uint32(0x00000FFF)
    base = u & np.uint32(0xFFFFF000)
    half = np.uint32(0x800)
    lsb = (u >> np.uint32(12)) & np.uint32(1)
    round_up = (low > half) | ((low == half) & (lsb == 1))
    return (base + np.where(round_up, np.uint32(0x1000), np.uint32(0))).view(
        np.float32
    )


def _pack_ww(W: np.ndarray) -> np.ndarray:
    # [64, 4096] -> [128, 32*128]: [p, k*128+e] = W_hi[e, k*128+p],
    #                              [p, k*128+64+e] = W_lo[e, k*128+p]
    wt = (
        W.astype(np.float32, copy=False)
        .T.reshape(N_CHUNK, 128, N_EXPERTS)
        .transpose(1, 0, 2)
    )  # [128, 32, 64]
    wh = _round_f32r(wt)
    wl = _round_f32r(wt - wh)
    ww = np.concatenate([wh.reshape(128, N_CHUNK, N_EXPERTS),
                         wl.reshape(128, N_CHUNK, N_EXPERTS)], axis=2)
    return np.ascontiguousarray(ww.reshape(128, N_CHUNK * 2 * N_EXPERTS))


def _make_in_maps(x: np.ndarray, W: np.ndarray) -> list:
    x = np.asarray(x, dtype=np.float32)
    ww = _pack_ww(W)
    return [
        {
            "xt": np.ascontiguousarray(x[c * TPC : (c + 1) * TPC].T),
            "ww": ww,
        }
        for c in range(N_CORES)
    ]


def kernel(x: np.ndarray, W: np.ndarray) -> np.ndarray:
    nc = _get_program()
    in_maps = _make_in_maps(x, W)
    res = run_bass_kernel_spmd(nc, in_maps, core_ids=list(range(N_CORES)))
    out = np.concatenate(
        [
            res.results[c]["idx"]
            .reshape(128, N_BLK, TOP_K)
            .transpose(1, 0, 2)
            .reshape(TPC, TOP_K)
            for c in range(N_CORES)
        ],
        axis=0,
    )
    return out.astype(np.int32)

